# revision 43
# baseline (speedup 1.0000x reference)
"""Trainium2 Bass kernel for nn_Block_68719476955 (dense transformer block).

Math: with H=1 the attention softmax is over a singleton axis, so
attn_prob == 1.0 exactly and the whole attention reduces to
x @ w_kv + b_kv (w_attn / b_attn / mask do not affect the output).

Folded main path: with A = I + w_kv and C = I - 11^T/E (mean-centering),
    LN1(x@A + b_kv) @ w_fc  ==  rstd ⊙ (x @ W1c + cb2) + c0
where W1c = A @ C @ (diag(g1) w_fc) is precomputed on host.  The E x E
kv matmul is needed only for the LN1 variance and runs in fp8e4
DoubleRow (2x column rate; variance tolerates fp8 noise).

The two big bf16 matmuls (x@W1c and u@w_mproj) run as one level of
Strassen (7/8 of the multiplies): the seven weight-side combinations are
precomputed on host; the seven activation-side combinations are built on
DVE; the seven products accumulate into the output quadrants in SBUF.
N=256 sub-matmuls stream at full rate (measured: LDWEIGHTS fully hides).

Distribution: pure data-parallel over the 8192 tokens across 8 cores
(1024 tokens/core), full weights on every core, no collectives.
"""

import numpy as np
import ml_dtypes
from contextlib import ExitStack

import concourse.bacc as bacc
import concourse.mybir as mybir
import concourse.tile as tile
from concourse.bass_utils import run_bass_kernel_spmd

P = 128
B, S, E = 4, 2048, 2048
H4 = 4 * E                 # 8192 mlp hidden
NCORES = 8
TOK = (B * S) // NCORES    # 1024 tokens per core
T = 512                    # token tile (2 per core)
T2 = T // 2                # Strassen N-half
NT = TOK // T
EO = E // P                # 16
EP = EO // 2               # 8 fp8 k-pairs; also fc Strassen K/2 chunks
FO = H4 // P               # 64
MF2 = 32                   # fc Strassen M/2 chunks (4096/128)
KM2 = 32                   # mp Strassen K/2 chunks (4096/128)
MM2 = 8                    # mp Strassen M/2 chunks (1024/128)
LN_EPS = 1e-5
F8MAX = 240.0

F32 = mybir.dt.float32
BF16 = mybir.dt.bfloat16
F8 = mybir.dt.float8e4
AF = mybir.ActivationFunctionType
ALU = mybir.AluOpType
DR = mybir.MatmulPerfMode.DoubleRow

# Strassen tables: product i uses stationary G_i and moving B_i, and
# accumulates (signed) into output quadrants.
TOUCH = {1: (("C11", 1), ("C22", 1)), 2: (("C21", 1), ("C22", -1)),
         3: (("C12", 1), ("C22", 1)), 4: (("C11", 1), ("C21", 1)),
         5: (("C11", -1), ("C12", 1)), 6: (("C22", 1),), 7: (("C11", 1),)}
BDEF = {1: ("11", "22", 1), 2: ("11", None, 0), 3: ("12", "22", -1),
        4: ("21", "11", -1), 5: ("22", None, 0), 6: ("11", "12", 1),
        7: ("21", "22", 1)}
FC_ORDER = (1, 7, 4, 5, 2, 3, 6)
MP_ORDER = (2, 4, 6, 1, 3, 5, 7)


def _plan(order):
    first, last = {}, {}
    for i in order:
        for q, _s in TOUCH[i]:
            first.setdefault(q, i)
            last[q] = i
    return first, last


FC_FIRST, FC_LAST = _plan(FC_ORDER)
MP_FIRST, MP_LAST = _plan(MP_ORDER)

_CACHED_NC = {}


def _build(has_cb2: bool, has_bmp: bool):
    nc = bacc.Bacc(None, target_bir_lowering=False)

    xf_d = nc.dram_tensor("xf", [E, TOK], F32, kind="ExternalInput")
    xb_d = nc.dram_tensor("xb", [E, TOK], BF16, kind="ExternalInput")
    x8_d = nc.dram_tensor("x8", [EP, P, 2, TOK], F8, kind="ExternalInput")
    a8_d = nc.dram_tensor("a8", [EO, P, EP, 2, P], F8, kind="ExternalInput")
    wfs_d = nc.dram_tensor("wfs", [7, MF2, P, EP, P], BF16, kind="ExternalInput")
    wms_d = nc.dram_tensor("wms", [7, MM2, P, KM2, P], BF16, kind="ExternalInput")
    bkv_d = nc.dram_tensor("bkv", [P, EO], F32, kind="ExternalInput")
    c0_d = nc.dram_tensor("c0", [P, FO], F32, kind="ExternalInput")
    cb2_d = nc.dram_tensor("cb2", [P, FO], F32, kind="ExternalInput")
    bmp_d = nc.dram_tensor("bmp", [P, EO], F32, kind="ExternalInput")
    g2_d = nc.dram_tensor("g2", [P, EO], F32, kind="ExternalInput")
    b2_d = nc.dram_tensor("b2", [P, EO], F32, kind="ExternalInput")
    sc1_d = nc.dram_tensor("sc1", [P, 1], F32, kind="ExternalInput")
    out_d = nc.dram_tensor("out", [E, TOK], BF16, kind="ExternalOutput")

    with tile.TileContext(nc) as tc, ExitStack() as ctx:
        # always-live pools only; the big fc/mp pools open after the
        # v-phase pools release their zones.
        consts = ctx.enter_context(tc.tile_pool(name="consts", bufs=1))
        xbp = ctx.enter_context(tc.tile_pool(name="xbp", bufs=1))
        xcp = ctx.enter_context(tc.tile_pool(name="xcp", bufs=2))
        tmp = ctx.enter_context(tc.tile_pool(name="tmp", bufs=3))
        sqp = ctx.enter_context(tc.tile_pool(name="sqp", bufs=12))
        stp = ctx.enter_context(tc.tile_pool(name="stp", bufs=1))
        bcp = ctx.enter_context(tc.tile_pool(name="bcp", bufs=2))
        r1p = ctx.enter_context(tc.tile_pool(name="r1p", bufs=10))
        psmm = ctx.enter_context(tc.tile_pool(name="psmm", bufs=4, space="PSUM"))
        psst = ctx.enter_context(tc.tile_pool(name="psst", bufs=2, space="PSUM"))

        # --- constants (gpsimd queue keeps Sync free for the weight stream) ---
        bkv_t = consts.tile([P, EO], F32)
        nc.gpsimd.dma_start(bkv_t[:], bkv_d[:, :])
        c0_t = consts.tile([P, FO], F32)
        nc.gpsimd.dma_start(c0_t[:], c0_d[:, :])
        cb2_t = consts.tile([P, FO], F32)
        nc.gpsimd.dma_start(cb2_t[:], cb2_d[:, :])
        bmp_t = consts.tile([P, EO], F32)
        nc.gpsimd.dma_start(bmp_t[:], bmp_d[:, :])
        g2_t = consts.tile([P, EO], F32)
        nc.gpsimd.dma_start(g2_t[:], g2_d[:, :])
        b2_t = consts.tile([P, EO], F32)
        nc.gpsimd.dma_start(b2_t[:], b2_d[:, :])
        sc1_t = consts.tile([P, 1], F32)
        nc.gpsimd.dma_start(sc1_t[:], sc1_d[:, :])
        ones_col = consts.tile([P, 1], BF16)
        nc.vector.memset(ones_col[:], 1.0)
        eps_t = consts.tile([1, 1], F32)
        nc.vector.memset(eps_t[:], LN_EPS)

        # warm the PE (HAM clock gate) while input DMAs are in flight
        warm_rhs = consts.tile([P, T], BF16)
        nc.vector.memset(warm_rhs[:], 1.0)
        warm_ps = psst.tile([1, T], F32, tag="pss")
        for _ in range(64):
            nc.tensor.matmul(warm_ps[:], lhsT=ones_col[:], rhs=warm_rhs[:],
                             start=True, stop=True)

        def tsl(t):
            return slice(t * T, (t + 1) * T)

        def make_stats():
            ps_sum = psst.tile([1, T], F32, tag="pss", name="ps_sum")
            ps_sq = psst.tile([1, T], F32, tag="psq", name="ps_sq")
            st = {"ps_sum": ps_sum, "ps_sq": ps_sq, "vals": [], "sqs": [], "g": 0}

            def push(val):
                sq = sqp.tile([P, T], BF16, tag="sq")
                nc.vector.tensor_mul(out=sq[:], in0=val, in1=val)
                st["vals"].append(val)
                st["sqs"].append(sq[:])

            def emit_quad():
                v0, v1, v2, v3 = st["vals"][:4]
                del st["vals"][:4]
                q0, q1, q2, q3 = st["sqs"][:4]
                del st["sqs"][:4]
                a = sqp.tile([P, T], BF16, tag="sq")
                nc.vector.tensor_add(out=a[:], in0=v0, in1=v1)
                b = sqp.tile([P, T], BF16, tag="sq")
                nc.vector.tensor_add(out=b[:], in0=v2, in1=v3)
                c = sqp.tile([P, T], BF16, tag="sq")
                nc.vector.tensor_add(out=c[:], in0=a[:], in1=b[:])
                qa = sqp.tile([P, T], BF16, tag="sq")
                nc.vector.tensor_add(out=qa[:], in0=q0, in1=q1)
                qb = sqp.tile([P, T], BF16, tag="sq")
                nc.vector.tensor_add(out=qb[:], in0=q2, in1=q3)
                qc = sqp.tile([P, T], BF16, tag="sq")
                nc.vector.tensor_add(out=qc[:], in0=qa[:], in1=qb[:])
                g = st["g"]
                st["g"] += 1
                nc.tensor.matmul(st["ps_sum"][:], lhsT=ones_col[:], rhs=c[:],
                                 start=(g == 0), stop=(g == 3))
                nc.tensor.matmul(st["ps_sq"][:], lhsT=ones_col[:], rhs=qc[:],
                                 start=(g == 0), stop=(g == 3))

            st["push"] = push
            st["emit_quad"] = emit_quad
            return st

        # ---------- phase V: fp8 DoubleRow x@A, LN1 stats only ----------
        def phase_v_both(x8s, a8p, r1p):
            st0 = make_stats()
            st1 = make_stats()
            for m in range(EO):
                if m in (4, 8, 12):
                    st0["emit_quad"]()
                    st1["emit_quad"]()
                a8t = a8p.tile([P, EP, 2, P], F8, tag="a8")
                nc.sync.dma_start(a8t[:], a8_d[m])
                for t in range(NT):
                    ps = psmm.tile([P, T], F32, tag="ps")
                    for j in range(EP):
                        nc.tensor.matmul(ps[:], lhsT=a8t[:, j],
                                         rhs=x8s[j][:, :, tsl(t)],
                                         start=(j == 0), stop=(j == EP - 1),
                                         perf_mode=DR)
                    r1c = r1p.tile([P, T], BF16, tag="r1")
                    nc.scalar.activation(r1c[:], ps[:], AF.Identity,
                                         bias=bkv_t[:, m:m + 1],
                                         scale=sc1_t[:, 0:1])
                    (st0 if t == 0 else st1)["push"](r1c[:])

            def finv():
                st0["emit_quad"]()
                st1["emit_quad"]()
            return st0, st1, finv

        def ln1_finalize(stats):
            """alpha = rsqrt(var + eps) -> [P,1,T] bf16 broadcast."""
            st = stp.tile([1, 3, T], F32, tag="st")
            nc.vector.tensor_scalar_mul(st[:, 0, :], stats["ps_sum"][:], 1.0 / E)
            nc.vector.tensor_scalar_mul(st[:, 1, :], stats["ps_sq"][:], 1.0 / E)
            nc.vector.tensor_mul(out=st[:, 2, :], in0=st[:, 0, :], in1=st[:, 0, :])
            nc.vector.tensor_tensor(st[:, 1, :], st[:, 1, :], st[:, 2, :],
                                    ALU.subtract)                          # var
            nc.scalar.activation(st[:, 2, :], st[:, 1, :], AF.Sqrt,
                                 bias=eps_t[:], scale=1.0)
            nc.vector.reciprocal(out=st[:, 2, :], in_=st[:, 2, :])         # rstd
            bcb = stp.tile([1, 1, T], BF16, tag="stb")
            nc.vector.tensor_copy(out=bcb[:], in_=st[:, 2:3, :])
            bc = bcp.tile([P, 1, T], BF16, tag="bc1")
            nc.gpsimd.partition_broadcast(bc[:], bcb[:])
            return bc

        # ---------- phase FC (Strassen): z = x@W1c, u = gelu(alpha*z+c0) ----
        def phase_fc(t, get_bc, hook=None, block_cb=None):
            u = up.tile([P, FO, T], BF16, tag="u")

            def xblk(q, k):
                kk = k if q[0] == "1" else EP + k
                base = t * T + (0 if q[1] == "1" else T2)
                return xbs[kk][:, base:base + T2]

            def uslice(q, r):
                row = r if q in ("C11", "C12") else MF2 + r
                col = slice(0, T2) if q in ("C11", "C21") else slice(T2, T)
                return u[:, row, col], row, col

            def gelu_pass(q):
                bc = get_bc()
                for r in range(MF2):
                    dst, row, col = uslice(q, r)
                    if has_cb2:
                        t1 = tmp.tile([P, T2], F32, tag="t1f")
                        nc.scalar.activation(t1[:], dst, AF.Identity,
                                             bias=cb2_t[:, row:row + 1],
                                             scale=1.0)
                        zin = t1[:]
                    else:
                        zin = dst
                    z1 = tmp.tile([P, T2], BF16, tag="z1")
                    nc.vector.tensor_mul(out=z1[:], in0=zin, in1=bc[:, 0, col])
                    nc.scalar.activation(dst, z1[:], AF.Gelu,
                                         bias=c0_t[:, row:row + 1], scale=1.0)

            blk = [0]
            for pos, i in enumerate(FC_ORDER):
                qa, qb, sgn = BDEF[i]
                combo = None
                if qb is not None:
                    combo = xcbp.tile([P, EP, T2], BF16, tag="bcf")
                    for k in range(EP):
                        if sgn > 0:
                            nc.vector.tensor_add(out=combo[:, k, :],
                                                 in0=xblk(qa, k), in1=xblk(qb, k))
                        else:
                            nc.vector.tensor_tensor(combo[:, k, :], xblk(qa, k),
                                                    xblk(qb, k), ALU.subtract)
                for r in range(MF2):
                    wt = wp.tile([P, EP, P], BF16, tag="wf")
                    weng = nc.sync if r % 2 == 0 else nc.scalar
                    weng.dma_start(wt[:], wfs_d[i - 1, r])
                    ps = psmm.tile([P, T2], F32, tag="ps")
                    for k in range(EP):
                        rhs = combo[:, k, :] if combo is not None else xblk(qa, k)
                        nc.tensor.matmul(ps[:], lhsT=wt[:, k, :], rhs=rhs,
                                         start=(k == 0), stop=(k == EP - 1))
                    if pos == 0 and r == 0 and hook is not None:
                        hook()
                    if block_cb is not None:
                        block_cb(blk[0])
                    blk[0] += 1
                    for q, s in TOUCH[i]:
                        dst, _row, _col = uslice(q, r)
                        if FC_FIRST[q] == i:
                            nc.scalar.activation(dst, ps[:], AF.Identity)
                        elif s > 0:
                            nc.vector.tensor_add(out=dst, in0=dst, in1=ps[:])
                        else:
                            nc.vector.tensor_tensor(dst, dst, ps[:], ALU.subtract)
                for q in ("C11", "C12", "C21", "C22"):
                    if FC_LAST[q] == i:
                        gelu_pass(q)
            return u

        # ---------- phase MP (Strassen): v = u@w_mproj + bmp + x ----------
        def phase_mp(t, u, hook=None):
            v2f = vp.tile([P, EO, T], BF16, tag="v")
            stats = make_stats()

            def ublk(q, kc):
                row = kc if q[0] == "1" else KM2 + kc
                col = slice(0, T2) if q[1] == "1" else slice(T2, T)
                return u[:, row, col]

            def vslice(q, r):
                row = r if q in ("C11", "C12") else MM2 + r
                col = slice(0, T2) if q in ("C11", "C21") else slice(T2, T)
                return v2f[:, row, col]

            def row_post(mo):
                xc = xcp.tile([P, T], F32, tag="xc")
                nc.gpsimd.dma_start(xc[:], xf_d[mo * P:(mo + 1) * P, tsl(t)])
                if has_bmp:
                    nc.scalar.activation(v2f[:, mo, :], v2f[:, mo, :],
                                         AF.Identity, bias=bmp_t[:, mo:mo + 1],
                                         scale=1.0)
                nc.vector.tensor_add(out=v2f[:, mo, :], in0=v2f[:, mo, :],
                                     in1=xc[:])
                stats["push"](v2f[:, mo, :])

            for pos, i in enumerate(MP_ORDER):
                qa, qb, sgn = BDEF[i]
                combos = None
                if qb is not None:
                    combos = []
                    for h in range(4):
                        cb = bcmp.tile([P, KM2 // 4, T2], BF16, tag="bcm")
                        for k in range(KM2 // 4):
                            kc = h * (KM2 // 4) + k
                            if sgn > 0:
                                nc.vector.tensor_add(out=cb[:, k, :],
                                                     in0=ublk(qa, kc),
                                                     in1=ublk(qb, kc))
                            else:
                                nc.vector.tensor_tensor(cb[:, k, :], ublk(qa, kc),
                                                        ublk(qb, kc), ALU.subtract)
                        combos.append(cb)
                h2 = KM2 // 2
                for r in range(MM2):
                    wta = wmp_p.tile([P, h2, P], BF16, tag="wma")
                    nc.sync.dma_start(wta[:], wms_d[i - 1, r][:, :h2, :])
                    wtb = wmp_p.tile([P, h2, P], BF16, tag="wmb")
                    nc.scalar.dma_start(wtb[:], wms_d[i - 1, r][:, h2:, :])
                    ps = psmm.tile([P, T2], F32, tag="ps")
                    for kc in range(KM2):
                        if combos is not None:
                            rhs = combos[kc // (KM2 // 4)][:, kc % (KM2 // 4), :]
                        else:
                            rhs = ublk(qa, kc)
                        wt = wta if kc < h2 else wtb
                        nc.tensor.matmul(ps[:], lhsT=wt[:, kc % h2, :], rhs=rhs,
                                         start=(kc == 0), stop=(kc == KM2 - 1))
                    if pos == 0 and r == 0 and hook is not None:
                        hook()
                    for q, s in TOUCH[i]:
                        dst = vslice(q, r)
                        if MP_FIRST[q] == i:
                            if s > 0:
                                nc.scalar.activation(dst, ps[:], AF.Identity)
                            else:
                                nc.vector.tensor_scalar_mul(dst, ps[:], -1.0)
                        elif s > 0:
                            nc.vector.tensor_add(out=dst, in0=dst, in1=ps[:])
                        else:
                            nc.vector.tensor_tensor(dst, dst, ps[:], ALU.subtract)
                if MP_LAST["C22"] == i:          # bottom rows complete
                    for mo in range(MM2, EO):
                        row_post(mo)
                if pos == len(MP_ORDER) - 2:     # PE cover for bottom quads
                    stats["emit_quad"]()
                    stats["emit_quad"]()
            for mo in range(MM2):                # top rows complete at end
                row_post(mo)

            def finish():
                stats["emit_quad"]()
                stats["emit_quad"]()
            return v2f, stats, finish

        def ln2_finalize(stats):
            """slots: [0]=mean*rstd, [1]=rstd -> [P,2,T] f32 broadcast."""
            st = stp.tile([1, 3, T], F32, tag="st")
            nc.vector.tensor_scalar_mul(st[:, 0, :], stats["ps_sum"][:], 1.0 / E)
            nc.vector.tensor_scalar_mul(st[:, 1, :], stats["ps_sq"][:], 1.0 / E)
            nc.vector.tensor_mul(out=st[:, 2, :], in0=st[:, 0, :], in1=st[:, 0, :])
            nc.vector.tensor_tensor(st[:, 1, :], st[:, 1, :], st[:, 2, :],
                                    ALU.subtract)
            nc.scalar.activation(st[:, 2, :], st[:, 1, :], AF.Sqrt,
                                 bias=eps_t[:], scale=1.0)
            nc.vector.reciprocal(out=st[:, 2, :], in_=st[:, 2, :])         # rstd
            nc.vector.tensor_mul(out=st[:, 1, :], in0=st[:, 0, :], in1=st[:, 2, :])
            stb2 = stp.tile([1, 2, T], BF16, tag="stb2")
            nc.vector.tensor_copy(out=stb2[:], in_=st[:, 1:3, :])
            bc = bcp.tile([P, 2, T], BF16, tag="bc2")
            nc.gpsimd.partition_broadcast(bc[:], stb2[:])
            return bc

        # ---------- phase C: final normalize + output (bf16) ----------
        def phase_c_chunk(t, v2f, bc, m, tail):
            eng = nc.gpsimd if (tail and m >= 12) else nc.vector
            t1 = tmp.tile([P, T], BF16, tag="t1")
            eng.tensor_mul(out=t1[:], in0=v2f[:, m, :], in1=bc[:, 1, :])
            eng.tensor_tensor(t1[:], t1[:], bc[:, 0, :], ALU.subtract)
            if tail:
                nc.scalar.activation(t1[:], t1[:], AF.Identity,
                                     bias=b2_t[:, m:m + 1],
                                     scale=g2_t[:, m:m + 1])
            else:
                nc.vector.tensor_scalar(t1[:], t1[:], g2_t[:, m:m + 1],
                                        b2_t[:, m:m + 1], ALU.mult, ALU.add)
            dma_eng = nc.sync if tail else nc.gpsimd
            dma_eng.dma_start(out_d[m * P:(m + 1) * P, tsl(t)], t1[:])

        def phase_c_out(t, v2f, bc, tail):
            for m in range(EO):
                phase_c_chunk(t, v2f, bc, m, tail)

        # ---------- emission ----------
        state = {}
        with ExitStack() as vctx:
            x8p = vctx.enter_context(tc.tile_pool(name="x8p", bufs=1))
            a8p = vctx.enter_context(tc.tile_pool(name="a8p", bufs=2))
            # fp8 x first (phase_v consumes it immediately)
            x8s = []
            for j in range(EP):
                xj = x8p.tile([P, 2, TOK], F8, tag=f"x8{j}")
                eng = nc.gpsimd if j % 2 == 0 else nc.scalar
                eng.dma_start(xj[:], x8_d[j])
                x8s.append(xj)
            xbs = []
            for k in range(EO):
                xk = xbp.tile([P, TOK], BF16, tag=f"xb{k}")
                eng = nc.gpsimd if k % 2 == 0 else nc.scalar
                eng.dma_start(xk[:], xb_d[k * P:(k + 1) * P, :])
                xbs.append(xk)
            s0, s1, finv = phase_v_both(x8s, a8p, r1p)

        # fc/mp pools open after the v pools released their zones
        wp = ctx.enter_context(tc.tile_pool(name="wp", bufs=4))
        wmp_p = ctx.enter_context(tc.tile_pool(name="wmp_p", bufs=2))
        xcbp = ctx.enter_context(tc.tile_pool(name="xcbp", bufs=2))
        bcmp = ctx.enter_context(tc.tile_pool(name="bcmp", bufs=4))
        up = ctx.enter_context(tc.tile_pool(name="up", bufs=1))
        vp = ctx.enter_context(tc.tile_pool(name="vp", bufs=1))

        def hook_fc0():
            finv()
            state["bc10"] = ln1_finalize(s0)
            state["bc11"] = ln1_finalize(s1)

        u0 = phase_fc(0, lambda: state["bc10"], hook=hook_fc0)
        v0, s20, fin20 = phase_mp(0, u0)

        def hook_fc1():
            fin20()
            state["bc20"] = ln2_finalize(s20)

        def cb_out0(blk):
            if blk % 12 == 0 and 1 <= blk // 12 <= 16:
                phase_c_chunk(0, v0, state["bc20"], blk // 12 - 1, tail=False)

        u1 = phase_fc(1, lambda: state["bc11"], hook=hook_fc1, block_cb=cb_out0)
        v1, s21, fin21 = phase_mp(1, u1)
        fin21()
        bc21 = ln2_finalize(s21)
        phase_c_out(1, v1, bc21, tail=True)

    nc.compile()
    return nc


def _get_nc(has_cb2: bool, has_bmp: bool):
    key = (has_cb2, has_bmp)
    if key not in _CACHED_NC:
        _CACHED_NC[key] = _build(has_cb2, has_bmp)
    return _CACHED_NC[key]


def _pow2_scale(amax):
    if amax <= 0:
        return 1.0
    return float(2.0 ** np.floor(np.log2(F8MAX / amax)))


def _strassen_weights(W, k2, m2, kchunks, mchunks):
    """W [K, M] -> 7 stacked retiled G_i, each [mchunks, P, kchunks, P]."""
    W11, W12 = W[:k2, :m2], W[:k2, m2:]
    W21, W22 = W[k2:, :m2], W[k2:, m2:]
    Gs = [W11 + W22, W12 + W22, W11, W22, W11 + W21, W12 - W11, W21 - W22]
    bf = ml_dtypes.bfloat16
    out = np.empty((7, mchunks, P, kchunks, P), bf)
    for idx, G in enumerate(Gs):
        G = np.asarray(G, np.float32).reshape(kchunks, P, mchunks, P)
        out[idx] = G.transpose(2, 1, 0, 3).astype(bf)
    return out


def _prep_inputs(x, w_kv, b_kv, w_fc, b_fc, w_mproj, b_mproj,
                 ln1_g, ln1_b, ln2_g, ln2_b):
    """Host-side fold + Strassen weight combos + shard. Returns
    (per-core input maps, has_cb2, has_bmp)."""
    bf = ml_dtypes.bfloat16
    f8 = ml_dtypes.float8_e4m3
    x_flat = np.ascontiguousarray(np.asarray(x, dtype=np.float32).reshape(B * S, E))
    w_kv = np.asarray(w_kv, np.float64)
    b_kv = np.asarray(b_kv, np.float64)
    w_fc = np.asarray(w_fc, np.float64)
    b_fc = np.asarray(b_fc, np.float64)
    g1 = np.asarray(ln1_g, np.float64)
    b1 = np.asarray(ln1_b, np.float64)

    # A = I + w_kv ; centered fold W1c = A @ (I - 11^T/E) @ diag(g1) @ w_fc
    A = w_kv.copy()
    A[np.diag_indices(E)] += 1.0
    Wg = w_fc * g1[:, None]
    Ac = A - A.sum(axis=1, keepdims=True) / E       # A @ C
    W1c = (Ac @ Wg).astype(np.float32)
    cb2 = ((b_kv - b_kv.mean()) @ Wg).astype(np.float32)     # b_kv @ C @ Wg
    c0 = (b1 @ w_fc + b_fc).astype(np.float32)
    bmp = np.asarray(b_mproj, np.float32)
    has_cb2 = bool(np.any(cb2 != 0.0))
    has_bmp = bool(np.any(bmp != 0.0))

    # fp8 quantization of A (stats path) and x
    s_A = _pow2_scale(np.abs(A).max())
    A8 = np.clip(A * s_A, -F8MAX, F8MAX).astype(f8)
    s_x = _pow2_scale(np.abs(x_flat).max())
    a8 = np.ascontiguousarray(
        A8.reshape(EP, 2, P, EO, P).transpose(3, 2, 0, 1, 4))

    wfs = _strassen_weights(W1c, E // 2, H4 // 2, EP, MF2)
    wms = _strassen_weights(np.asarray(w_mproj, np.float64),
                            H4 // 2, E // 2, KM2, MM2)

    def p2d(v):  # [n*P] -> [P, n] with chunk o in column o
        v = np.asarray(v, np.float32)
        return np.ascontiguousarray(v.reshape(-1, P).T)

    shared = {
        "a8": a8, "wfs": wfs, "wms": wms,
        "bkv": p2d(b_kv.astype(np.float32)), "c0": p2d(c0), "cb2": p2d(cb2),
        "bmp": p2d(bmp),
        "g2": p2d(ln2_g), "b2": p2d(ln2_b),
        "sc1": np.full((P, 1), 1.0 / (s_A * s_x), np.float32),
    }
    in_maps = []
    for c in range(NCORES):
        xT = np.ascontiguousarray(x_flat[c * TOK:(c + 1) * TOK].T)  # [E, TOK] f32
        x8c = np.clip(xT * s_x, -F8MAX, F8MAX).astype(f8)           # [E, TOK]
        x8c = np.ascontiguousarray(
            x8c.reshape(EP, 2, P, TOK).transpose(0, 2, 1, 3))
        in_maps.append({"xf": xT, "xb": xT.astype(bf), "x8": x8c, **shared})
    return in_maps, has_cb2, has_bmp


def _run(inputs, trace=False):
    in_maps, has_cb2, has_bmp = _prep_inputs(
        inputs["x"], inputs["w_kv"], inputs["b_kv"], inputs["w_fc"],
        inputs["b_fc"], inputs["w_mproj"], inputs["b_mproj"],
        inputs["ln1_g"], inputs["ln1_b"], inputs["ln2_g"], inputs["ln2_b"])
    nc = _get_nc(has_cb2, has_bmp)
    res = run_bass_kernel_spmd(nc, in_maps, core_ids=list(range(NCORES)),
                               trace=trace)
    outs = [np.asarray(res.results[c]["out"]).astype(np.float32).T
            for c in range(NCORES)]
    full = np.concatenate(outs, axis=0).reshape(B, S, E)
    return full, res


def kernel(**inputs) -> np.ndarray:
    out, _ = _run(inputs, trace=False)
    return out


# revision 47
# speedup vs baseline: 1.0766x; 1.0766x over previous
"""Trainium2 Bass kernel for nn_Block_68719476955 (dense transformer block).

Math: with H=1 the attention softmax is over a singleton axis, so
attn_prob == 1.0 exactly and the whole attention reduces to
x @ w_kv + b_kv (w_attn / b_attn / mask do not affect the output).

Folded main path: with A = I + w_kv and C = I - 11^T/E (mean-centering),
    LN1(x@A + b_kv) @ w_fc  ==  rstd ⊙ (x @ W1c + cb2) + c0
where W1c = A @ C @ (diag(g1) w_fc) is precomputed on host.  The E x E
kv matmul is needed only for the LN1 variance and runs in fp8e4
DoubleRow (2x column rate; variance tolerates fp8 noise).

The two big bf16 matmuls (x@W1c and u@w_mproj) run as one level of
Strassen (7/8 of the multiplies): the seven weight-side combinations are
precomputed on host; the seven activation-side combinations are built on
DVE; the seven products accumulate into the output quadrants in SBUF.
N=256 sub-matmuls stream at full rate (measured: LDWEIGHTS fully hides).

Distribution: pure data-parallel over the 8192 tokens across 8 cores
(1024 tokens/core), full weights on every core, no collectives.
"""

import numpy as np
import ml_dtypes
from contextlib import ExitStack

import concourse.bacc as bacc
import concourse.mybir as mybir
import concourse.tile as tile
from concourse.bass_utils import run_bass_kernel_spmd

P = 128
B, S, E = 4, 2048, 2048
H4 = 4 * E                 # 8192 mlp hidden
NCORES = 8
TOK = (B * S) // NCORES    # 1024 tokens per core
T = 512                    # token tile (2 per core)
T2 = T // 2                # Strassen N-half
NT = TOK // T
EO = E // P                # 16
EP = EO // 2               # 8 fp8 k-pairs; also fc Strassen K/2 chunks
FO = H4 // P               # 64
MF2 = 32                   # fc Strassen M/2 chunks (4096/128)
KM2 = 32                   # mp Strassen K/2 chunks (4096/128)
MM2 = 8                    # mp Strassen M/2 chunks (1024/128)
LN_EPS = 1e-5
F8MAX = 240.0

F32 = mybir.dt.float32
BF16 = mybir.dt.bfloat16
F8 = mybir.dt.float8e4
AF = mybir.ActivationFunctionType
ALU = mybir.AluOpType
DR = mybir.MatmulPerfMode.DoubleRow

# Strassen tables: product i uses stationary G_i and moving B_i, and
# accumulates (signed) into output quadrants.
TOUCH = {1: (("C11", 1), ("C22", 1)), 2: (("C21", 1), ("C22", -1)),
         3: (("C12", 1), ("C22", 1)), 4: (("C11", 1), ("C21", 1)),
         5: (("C11", -1), ("C12", 1)), 6: (("C22", 1),), 7: (("C11", 1),)}
BDEF = {1: ("11", "22", 1), 2: ("11", None, 0), 3: ("12", "22", -1),
        4: ("21", "11", -1), 5: ("22", None, 0), 6: ("11", "12", 1),
        7: ("21", "22", 1)}
FC_ORDER = (1, 7, 4, 5, 2, 3, 6)
MP_ORDER = (2, 4, 6, 1, 3, 5, 7)


def _plan(order):
    first, last = {}, {}
    for i in order:
        for q, _s in TOUCH[i]:
            first.setdefault(q, i)
            last[q] = i
    return first, last


FC_FIRST, FC_LAST = _plan(FC_ORDER)
MP_FIRST, MP_LAST = _plan(MP_ORDER)

_CACHED_NC = {}


def _build(has_cb2: bool, has_bmp: bool):
    nc = bacc.Bacc(None, target_bir_lowering=False)

    xf_d = nc.dram_tensor("xf", [E, TOK], F32, kind="ExternalInput")
    xb_d = nc.dram_tensor("xb", [E, TOK], BF16, kind="ExternalInput")
    x8_d = nc.dram_tensor("x8", [EP, P, 2, TOK], F8, kind="ExternalInput")
    a8_d = nc.dram_tensor("a8", [EO, P, EP, 2, P], F8, kind="ExternalInput")
    wfs_d = nc.dram_tensor("wfs", [7, MF2, P, EP, P], BF16, kind="ExternalInput")
    wms_d = nc.dram_tensor("wms", [7, MM2, P, KM2, P], BF16, kind="ExternalInput")
    bkv_d = nc.dram_tensor("bkv", [P, EO], F32, kind="ExternalInput")
    c0_d = nc.dram_tensor("c0", [P, FO], F32, kind="ExternalInput")
    cb2_d = nc.dram_tensor("cb2", [P, FO], F32, kind="ExternalInput")
    bmp_d = nc.dram_tensor("bmp", [P, EO], F32, kind="ExternalInput")
    g2_d = nc.dram_tensor("g2", [P, EO], F32, kind="ExternalInput")
    b2_d = nc.dram_tensor("b2", [P, EO], F32, kind="ExternalInput")
    sc1_d = nc.dram_tensor("sc1", [P, 1], F32, kind="ExternalInput")
    out_d = nc.dram_tensor("out", [E, TOK], BF16, kind="ExternalOutput")

    with tile.TileContext(nc) as tc, ExitStack() as ctx:
        # always-live pools only; the big fc/mp pools open after the
        # v-phase pools release their zones.
        consts = ctx.enter_context(tc.tile_pool(name="consts", bufs=1))
        xbp = ctx.enter_context(tc.tile_pool(name="xbp", bufs=1))
        xcp = ctx.enter_context(tc.tile_pool(name="xcp", bufs=2))
        tmp = ctx.enter_context(tc.tile_pool(name="tmp", bufs=3))
        sqp = ctx.enter_context(tc.tile_pool(name="sqp", bufs=12))
        stp = ctx.enter_context(tc.tile_pool(name="stp", bufs=1))
        bcp = ctx.enter_context(tc.tile_pool(name="bcp", bufs=2))
        r1p = ctx.enter_context(tc.tile_pool(name="r1p", bufs=10))
        psmm = ctx.enter_context(tc.tile_pool(name="psmm", bufs=4, space="PSUM"))
        psst = ctx.enter_context(tc.tile_pool(name="psst", bufs=2, space="PSUM"))

        # --- constants (gpsimd queue keeps Sync free for the weight stream) ---
        bkv_t = consts.tile([P, EO], F32)
        nc.gpsimd.dma_start(bkv_t[:], bkv_d[:, :])
        c0_t = consts.tile([P, FO], F32)
        nc.gpsimd.dma_start(c0_t[:], c0_d[:, :])
        cb2_t = consts.tile([P, FO], F32)
        nc.gpsimd.dma_start(cb2_t[:], cb2_d[:, :])
        bmp_t = consts.tile([P, EO], F32)
        nc.gpsimd.dma_start(bmp_t[:], bmp_d[:, :])
        g2_t = consts.tile([P, EO], F32)
        nc.gpsimd.dma_start(g2_t[:], g2_d[:, :])
        b2_t = consts.tile([P, EO], F32)
        nc.gpsimd.dma_start(b2_t[:], b2_d[:, :])
        sc1_t = consts.tile([P, 1], F32)
        nc.gpsimd.dma_start(sc1_t[:], sc1_d[:, :])
        ones_col = consts.tile([P, 1], BF16)
        nc.vector.memset(ones_col[:], 1.0)
        eps_t = consts.tile([1, 1], F32)
        nc.vector.memset(eps_t[:], LN_EPS)

        # warm the PE (HAM clock gate) while input DMAs are in flight
        warm_rhs = consts.tile([P, T], BF16)
        nc.vector.memset(warm_rhs[:], 1.0)
        warm_ps = psst.tile([1, T], F32, tag="pss")
        for _ in range(64):
            nc.tensor.matmul(warm_ps[:], lhsT=ones_col[:], rhs=warm_rhs[:],
                             start=True, stop=True)

        def tsl(t):
            return slice(t * T, (t + 1) * T)

        def make_stats():
            ps_sum = psst.tile([1, T], F32, tag="pss", name="ps_sum")
            ps_sq = psst.tile([1, T], F32, tag="psq", name="ps_sq")
            st = {"ps_sum": ps_sum, "ps_sq": ps_sq, "vals": [], "sqs": [], "g": 0}

            def push(val):
                sq = sqp.tile([P, T], BF16, tag="sq")
                nc.vector.tensor_mul(out=sq[:], in0=val, in1=val)
                st["vals"].append(val)
                st["sqs"].append(sq[:])

            def emit_quad():
                v0, v1, v2, v3 = st["vals"][:4]
                del st["vals"][:4]
                q0, q1, q2, q3 = st["sqs"][:4]
                del st["sqs"][:4]
                a = sqp.tile([P, T], BF16, tag="sq")
                nc.vector.tensor_add(out=a[:], in0=v0, in1=v1)
                b = sqp.tile([P, T], BF16, tag="sq")
                nc.vector.tensor_add(out=b[:], in0=v2, in1=v3)
                c = sqp.tile([P, T], BF16, tag="sq")
                nc.vector.tensor_add(out=c[:], in0=a[:], in1=b[:])
                qa = sqp.tile([P, T], BF16, tag="sq")
                nc.vector.tensor_add(out=qa[:], in0=q0, in1=q1)
                qb = sqp.tile([P, T], BF16, tag="sq")
                nc.vector.tensor_add(out=qb[:], in0=q2, in1=q3)
                qc = sqp.tile([P, T], BF16, tag="sq")
                nc.vector.tensor_add(out=qc[:], in0=qa[:], in1=qb[:])
                g = st["g"]
                st["g"] += 1
                nc.tensor.matmul(st["ps_sum"][:], lhsT=ones_col[:], rhs=c[:],
                                 start=(g == 0), stop=(g == 3))
                nc.tensor.matmul(st["ps_sq"][:], lhsT=ones_col[:], rhs=qc[:],
                                 start=(g == 0), stop=(g == 3))

            st["push"] = push
            st["emit_quad"] = emit_quad
            return st

        # ---------- phase V: fp8 DoubleRow x@A, LN1 stats only ----------
        def phase_v_both(x8s, a8p, r1p):
            st0 = make_stats()
            st1 = make_stats()
            for m in range(EO):
                if m in (4, 8, 12):
                    st0["emit_quad"]()
                    st1["emit_quad"]()
                a8t = a8p.tile([P, EP, 2, P], F8, tag="a8")
                nc.sync.dma_start(a8t[:], a8_d[m])
                for t in range(NT):
                    ps = psmm.tile([P, T], F32, tag="ps")
                    for j in range(EP):
                        nc.tensor.matmul(ps[:], lhsT=a8t[:, j],
                                         rhs=x8s[j][:, :, tsl(t)],
                                         start=(j == 0), stop=(j == EP - 1),
                                         perf_mode=DR)
                    r1c = r1p.tile([P, T], BF16, tag="r1")
                    nc.scalar.activation(r1c[:], ps[:], AF.Identity,
                                         bias=bkv_t[:, m:m + 1],
                                         scale=sc1_t[:, 0:1])
                    (st0 if t == 0 else st1)["push"](r1c[:])

            def finv():
                st0["emit_quad"]()
                st1["emit_quad"]()
            return st0, st1, finv

        def ln1_finalize(stats):
            """alpha = rsqrt(var + eps) -> [P,1,T] bf16 broadcast."""
            st = stp.tile([1, 3, T], F32, tag="st")
            nc.vector.tensor_scalar_mul(st[:, 0, :], stats["ps_sum"][:], 1.0 / E)
            nc.vector.tensor_scalar_mul(st[:, 1, :], stats["ps_sq"][:], 1.0 / E)
            nc.vector.tensor_mul(out=st[:, 2, :], in0=st[:, 0, :], in1=st[:, 0, :])
            nc.vector.tensor_tensor(st[:, 1, :], st[:, 1, :], st[:, 2, :],
                                    ALU.subtract)                          # var
            nc.scalar.activation(st[:, 2, :], st[:, 1, :], AF.Sqrt,
                                 bias=eps_t[:], scale=1.0)
            nc.vector.reciprocal(out=st[:, 2, :], in_=st[:, 2, :])         # rstd
            bcb = stp.tile([1, 1, T], BF16, tag="stb")
            nc.vector.tensor_copy(out=bcb[:], in_=st[:, 2:3, :])
            bc = bcp.tile([P, 1, T], BF16, tag="bc1")
            nc.gpsimd.partition_broadcast(bc[:], bcb[:])
            return bc

        # ---------- phase FC (Strassen): z = x@W1c, u = gelu(alpha*z+c0) ----
        # pending: deferred per-(quadrant,row) gelu items drained a couple
        # per MM group (keeps ACT/DVE bursts off the PE critical path);
        # leftovers drain into the following mp phase's groups.
        def phase_fc(t, get_bc, pending, hook=None, block_cb=None):
            u = up.tile([P, FO, T], BF16, tag="u")

            def xblk(q, k):
                kk = k if q[0] == "1" else EP + k
                base = t * T + (0 if q[1] == "1" else T2)
                return xbs[kk][:, base:base + T2]

            def uslice(q, r):
                row = r if q in ("C11", "C12") else MF2 + r
                col = slice(0, T2) if q in ("C11", "C21") else slice(T2, T)
                return u[:, row, col], row, col

            def gelu_item(q, r):
                def emit():
                    bc = get_bc()
                    dst, row, col = uslice(q, r)
                    if has_cb2:
                        t1 = tmp.tile([P, T2], F32, tag="t1f")
                        nc.scalar.activation(t1[:], dst, AF.Identity,
                                             bias=cb2_t[:, row:row + 1],
                                             scale=1.0)
                        zin = t1[:]
                    else:
                        zin = dst
                    z1 = tmp.tile([P, T2], BF16, tag="z1")
                    nc.vector.tensor_mul(out=z1[:], in0=zin, in1=bc[:, 0, col])
                    nc.scalar.activation(dst, z1[:], AF.Gelu,
                                         bias=c0_t[:, row:row + 1], scale=1.0)
                return emit

            blk = [0]
            for pos, i in enumerate(FC_ORDER):
                qa, qb, sgn = BDEF[i]
                combo = None
                if qb is not None:
                    combo = xcbp.tile([P, EP, T2], BF16, tag="bcf")
                    for k in range(EP):
                        if sgn > 0:
                            nc.vector.tensor_add(out=combo[:, k, :],
                                                 in0=xblk(qa, k), in1=xblk(qb, k))
                        else:
                            nc.vector.tensor_tensor(combo[:, k, :], xblk(qa, k),
                                                    xblk(qb, k), ALU.subtract)
                for r in range(MF2):
                    wt = wp.tile([P, EP, P], BF16, tag="wf")
                    weng = nc.sync if r % 2 == 0 else nc.scalar
                    weng.dma_start(wt[:], wfs_d[i - 1, r])
                    ps = psmm.tile([P, T2], F32, tag="ps")
                    for k in range(EP):
                        rhs = combo[:, k, :] if combo is not None else xblk(qa, k)
                        nc.tensor.matmul(ps[:], lhsT=wt[:, k, :], rhs=rhs,
                                         start=(k == 0), stop=(k == EP - 1))
                    if hook is not None:
                        hook(blk[0])
                    if block_cb is not None:
                        block_cb(blk[0])
                    blk[0] += 1
                    for q, s in TOUCH[i]:
                        dst, _row, _col = uslice(q, r)
                        if FC_FIRST[q] == i:
                            nc.scalar.activation(dst, ps[:], AF.Identity)
                        elif s > 0:
                            nc.vector.tensor_add(out=dst, in0=dst, in1=ps[:])
                        else:
                            nc.vector.tensor_tensor(dst, dst, ps[:], ALU.subtract)
                    for _ in range(2):
                        if pending:
                            pending.pop(0)()
                for q in ("C11", "C12", "C21", "C22"):
                    if FC_LAST[q] == i:
                        for r in range(MF2):
                            pending.append(gelu_item(q, r))
            return u

        # ---------- phase MP (Strassen): v = u@w_mproj + bmp + x ----------
        def phase_mp(t, u, pending, hook=None):
            v2f = vp.tile([P, EO, T], BF16, tag="v")
            stats = make_stats()

            def ublk(q, kc):
                row = kc if q[0] == "1" else KM2 + kc
                col = slice(0, T2) if q[1] == "1" else slice(T2, T)
                return u[:, row, col]

            def vslice(q, r):
                row = r if q in ("C11", "C12") else MM2 + r
                col = slice(0, T2) if q in ("C11", "C21") else slice(T2, T)
                return v2f[:, row, col]

            def row_post(mo):
                xc = xcp.tile([P, T], F32, tag="xc")
                nc.gpsimd.dma_start(xc[:], xf_d[mo * P:(mo + 1) * P, tsl(t)])
                if has_bmp:
                    nc.scalar.activation(v2f[:, mo, :], v2f[:, mo, :],
                                         AF.Identity, bias=bmp_t[:, mo:mo + 1],
                                         scale=1.0)
                nc.vector.tensor_add(out=v2f[:, mo, :], in0=v2f[:, mo, :],
                                     in1=xc[:])
                stats["push"](v2f[:, mo, :])

            for pos, i in enumerate(MP_ORDER):
                qa, qb, sgn = BDEF[i]
                combos = None
                if qb is not None:
                    combos = []
                    for h in range(4):
                        cb = bcmp.tile([P, KM2 // 4, T2], BF16, tag="bcm")
                        for k in range(KM2 // 4):
                            kc = h * (KM2 // 4) + k
                            if sgn > 0:
                                nc.vector.tensor_add(out=cb[:, k, :],
                                                     in0=ublk(qa, kc),
                                                     in1=ublk(qb, kc))
                            else:
                                nc.vector.tensor_tensor(cb[:, k, :], ublk(qa, kc),
                                                        ublk(qb, kc), ALU.subtract)
                        combos.append(cb)
                h2 = KM2 // 2
                for r in range(MM2):
                    wta = wmp_p.tile([P, h2, P], BF16, tag="wma")
                    nc.sync.dma_start(wta[:], wms_d[i - 1, r][:, :h2, :])
                    wtb = wmp_p.tile([P, h2, P], BF16, tag="wmb")
                    nc.scalar.dma_start(wtb[:], wms_d[i - 1, r][:, h2:, :])
                    ps = psmm.tile([P, T2], F32, tag="ps")
                    for kc in range(KM2):
                        if combos is not None:
                            rhs = combos[kc // (KM2 // 4)][:, kc % (KM2 // 4), :]
                        else:
                            rhs = ublk(qa, kc)
                        wt = wta if kc < h2 else wtb
                        nc.tensor.matmul(ps[:], lhsT=wt[:, kc % h2, :], rhs=rhs,
                                         start=(kc == 0), stop=(kc == KM2 - 1))
                    if pos == 0 and r == 0 and hook is not None:
                        hook()
                    for _ in range(4):
                        if pending:
                            pending.pop(0)()
                    for q, s in TOUCH[i]:
                        dst = vslice(q, r)
                        if MP_FIRST[q] == i:
                            if s > 0:
                                nc.scalar.activation(dst, ps[:], AF.Identity)
                            else:
                                nc.vector.tensor_scalar_mul(dst, ps[:], -1.0)
                        elif s > 0:
                            nc.vector.tensor_add(out=dst, in0=dst, in1=ps[:])
                        else:
                            nc.vector.tensor_tensor(dst, dst, ps[:], ALU.subtract)
                if MP_LAST["C22"] == i:          # bottom rows complete
                    for mo in range(MM2, EO):
                        row_post(mo)
                if pos == len(MP_ORDER) - 2:     # PE cover for bottom quads
                    stats["emit_quad"]()
                    stats["emit_quad"]()
            for mo in range(MM2):                # top rows complete at end
                row_post(mo)

            def finish():
                stats["emit_quad"]()
                stats["emit_quad"]()
            return v2f, stats, finish

        def ln2_finalize(stats):
            """slots: [0]=mean*rstd, [1]=rstd -> [P,2,T] f32 broadcast."""
            st = stp.tile([1, 3, T], F32, tag="st")
            nc.vector.tensor_scalar_mul(st[:, 0, :], stats["ps_sum"][:], 1.0 / E)
            nc.vector.tensor_scalar_mul(st[:, 1, :], stats["ps_sq"][:], 1.0 / E)
            nc.vector.tensor_mul(out=st[:, 2, :], in0=st[:, 0, :], in1=st[:, 0, :])
            nc.vector.tensor_tensor(st[:, 1, :], st[:, 1, :], st[:, 2, :],
                                    ALU.subtract)
            nc.scalar.activation(st[:, 2, :], st[:, 1, :], AF.Sqrt,
                                 bias=eps_t[:], scale=1.0)
            nc.vector.reciprocal(out=st[:, 2, :], in_=st[:, 2, :])         # rstd
            nc.vector.tensor_mul(out=st[:, 1, :], in0=st[:, 0, :], in1=st[:, 2, :])
            stb2 = stp.tile([1, 2, T], BF16, tag="stb2")
            nc.vector.tensor_copy(out=stb2[:], in_=st[:, 1:3, :])
            bc = bcp.tile([P, 2, T], BF16, tag="bc2")
            nc.gpsimd.partition_broadcast(bc[:], stb2[:])
            return bc

        # ---------- phase C: final normalize + output (bf16) ----------
        def phase_c_chunk(t, v2f, bc, m, tail):
            eng = nc.gpsimd if (tail and m >= 12) else nc.vector
            t1 = tmp.tile([P, T], BF16, tag="t1")
            eng.tensor_mul(out=t1[:], in0=v2f[:, m, :], in1=bc[:, 1, :])
            eng.tensor_tensor(t1[:], t1[:], bc[:, 0, :], ALU.subtract)
            if tail:
                nc.scalar.activation(t1[:], t1[:], AF.Identity,
                                     bias=b2_t[:, m:m + 1],
                                     scale=g2_t[:, m:m + 1])
            else:
                nc.vector.tensor_scalar(t1[:], t1[:], g2_t[:, m:m + 1],
                                        b2_t[:, m:m + 1], ALU.mult, ALU.add)
            dma_eng = nc.sync if tail else nc.gpsimd
            dma_eng.dma_start(out_d[m * P:(m + 1) * P, tsl(t)], t1[:])

        def phase_c_out(t, v2f, bc, tail):
            for m in range(EO):
                phase_c_chunk(t, v2f, bc, m, tail)

        # ---------- emission ----------
        state = {}
        with ExitStack() as vctx:
            x8p = vctx.enter_context(tc.tile_pool(name="x8p", bufs=1))
            a8p = vctx.enter_context(tc.tile_pool(name="a8p", bufs=2))
            # fp8 x first (phase_v consumes it immediately)
            x8s = []
            for j in range(EP):
                xj = x8p.tile([P, 2, TOK], F8, tag=f"x8{j}")
                eng = nc.gpsimd if j % 2 == 0 else nc.scalar
                eng.dma_start(xj[:], x8_d[j])
                x8s.append(xj)
            xbs = []
            for k in range(EO):
                xk = xbp.tile([P, TOK], BF16, tag=f"xb{k}")
                eng = nc.gpsimd if k % 2 == 0 else nc.scalar
                eng.dma_start(xk[:], xb_d[k * P:(k + 1) * P, :])
                xbs.append(xk)
            s0, s1, finv = phase_v_both(x8s, a8p, r1p)

        # fc/mp pools open after the v pools released their zones
        wp = ctx.enter_context(tc.tile_pool(name="wp", bufs=4))
        wmp_p = ctx.enter_context(tc.tile_pool(name="wmp_p", bufs=2))
        xcbp = ctx.enter_context(tc.tile_pool(name="xcbp", bufs=2))
        bcmp = ctx.enter_context(tc.tile_pool(name="bcmp", bufs=4))
        up = ctx.enter_context(tc.tile_pool(name="up", bufs=1))
        vp = ctx.enter_context(tc.tile_pool(name="vp", bufs=1))

        def hook_fc0(blk):
            if blk == 1:
                finv()
            elif blk == 3:
                state["bc10"] = ln1_finalize(s0)
            elif blk == 6:
                state["bc11"] = ln1_finalize(s1)

        pend0, pend1 = [], []
        u0 = phase_fc(0, lambda: state["bc10"], pend0, hook=hook_fc0)
        v0, s20, fin20 = phase_mp(0, u0, pend0)

        def hook_fc1(blk):
            if blk == 0:
                fin20()
            elif blk == 2:
                state["bc20"] = ln2_finalize(s20)

        def cb_out0(blk):
            if blk % 12 == 0 and 1 <= blk // 12 <= 16:
                phase_c_chunk(0, v0, state["bc20"], blk // 12 - 1, tail=False)

        u1 = phase_fc(1, lambda: state["bc11"], pend1, hook=hook_fc1,
                      block_cb=cb_out0)
        v1, s21, fin21 = phase_mp(1, u1, pend1)
        fin21()
        bc21 = ln2_finalize(s21)
        phase_c_out(1, v1, bc21, tail=True)

    nc.compile()
    return nc


def _get_nc(has_cb2: bool, has_bmp: bool):
    key = (has_cb2, has_bmp)
    if key not in _CACHED_NC:
        _CACHED_NC[key] = _build(has_cb2, has_bmp)
    return _CACHED_NC[key]


def _pow2_scale(amax):
    if amax <= 0:
        return 1.0
    return float(2.0 ** np.floor(np.log2(F8MAX / amax)))


def _strassen_weights(W, k2, m2, kchunks, mchunks):
    """W [K, M] -> 7 stacked retiled G_i, each [mchunks, P, kchunks, P]."""
    W11, W12 = W[:k2, :m2], W[:k2, m2:]
    W21, W22 = W[k2:, :m2], W[k2:, m2:]
    Gs = [W11 + W22, W12 + W22, W11, W22, W11 + W21, W12 - W11, W21 - W22]
    bf = ml_dtypes.bfloat16
    out = np.empty((7, mchunks, P, kchunks, P), bf)
    for idx, G in enumerate(Gs):
        G = np.asarray(G, np.float32).reshape(kchunks, P, mchunks, P)
        out[idx] = G.transpose(2, 1, 0, 3).astype(bf)
    return out


def _prep_inputs(x, w_kv, b_kv, w_fc, b_fc, w_mproj, b_mproj,
                 ln1_g, ln1_b, ln2_g, ln2_b):
    """Host-side fold + Strassen weight combos + shard. Returns
    (per-core input maps, has_cb2, has_bmp)."""
    bf = ml_dtypes.bfloat16
    f8 = ml_dtypes.float8_e4m3
    x_flat = np.ascontiguousarray(np.asarray(x, dtype=np.float32).reshape(B * S, E))
    w_kv = np.asarray(w_kv, np.float64)
    b_kv = np.asarray(b_kv, np.float64)
    w_fc = np.asarray(w_fc, np.float64)
    b_fc = np.asarray(b_fc, np.float64)
    g1 = np.asarray(ln1_g, np.float64)
    b1 = np.asarray(ln1_b, np.float64)

    # A = I + w_kv ; centered fold W1c = A @ (I - 11^T/E) @ diag(g1) @ w_fc
    A = w_kv.copy()
    A[np.diag_indices(E)] += 1.0
    Wg = w_fc * g1[:, None]
    Ac = A - A.sum(axis=1, keepdims=True) / E       # A @ C
    W1c = (Ac @ Wg).astype(np.float32)
    cb2 = ((b_kv - b_kv.mean()) @ Wg).astype(np.float32)     # b_kv @ C @ Wg
    c0 = (b1 @ w_fc + b_fc).astype(np.float32)
    bmp = np.asarray(b_mproj, np.float32)
    has_cb2 = bool(np.any(cb2 != 0.0))
    has_bmp = bool(np.any(bmp != 0.0))

    # fp8 quantization of A (stats path) and x
    s_A = _pow2_scale(np.abs(A).max())
    A8 = np.clip(A * s_A, -F8MAX, F8MAX).astype(f8)
    s_x = _pow2_scale(np.abs(x_flat).max())
    a8 = np.ascontiguousarray(
        A8.reshape(EP, 2, P, EO, P).transpose(3, 2, 0, 1, 4))

    wfs = _strassen_weights(W1c, E // 2, H4 // 2, EP, MF2)
    wms = _strassen_weights(np.asarray(w_mproj, np.float64),
                            H4 // 2, E // 2, KM2, MM2)

    def p2d(v):  # [n*P] -> [P, n] with chunk o in column o
        v = np.asarray(v, np.float32)
        return np.ascontiguousarray(v.reshape(-1, P).T)

    shared = {
        "a8": a8, "wfs": wfs, "wms": wms,
        "bkv": p2d(b_kv.astype(np.float32)), "c0": p2d(c0), "cb2": p2d(cb2),
        "bmp": p2d(bmp),
        "g2": p2d(ln2_g), "b2": p2d(ln2_b),
        "sc1": np.full((P, 1), 1.0 / (s_A * s_x), np.float32),
    }
    in_maps = []
    for c in range(NCORES):
        xT = np.ascontiguousarray(x_flat[c * TOK:(c + 1) * TOK].T)  # [E, TOK] f32
        x8c = np.clip(xT * s_x, -F8MAX, F8MAX).astype(f8)           # [E, TOK]
        x8c = np.ascontiguousarray(
            x8c.reshape(EP, 2, P, TOK).transpose(0, 2, 1, 3))
        in_maps.append({"xf": xT, "xb": xT.astype(bf), "x8": x8c, **shared})
    return in_maps, has_cb2, has_bmp


def _run(inputs, trace=False):
    in_maps, has_cb2, has_bmp = _prep_inputs(
        inputs["x"], inputs["w_kv"], inputs["b_kv"], inputs["w_fc"],
        inputs["b_fc"], inputs["w_mproj"], inputs["b_mproj"],
        inputs["ln1_g"], inputs["ln1_b"], inputs["ln2_g"], inputs["ln2_b"])
    nc = _get_nc(has_cb2, has_bmp)
    res = run_bass_kernel_spmd(nc, in_maps, core_ids=list(range(NCORES)),
                               trace=trace)
    outs = [np.asarray(res.results[c]["out"]).astype(np.float32).T
            for c in range(NCORES)]
    full = np.concatenate(outs, axis=0).reshape(B, S, E)
    return full, res


def kernel(**inputs) -> np.ndarray:
    out, _ = _run(inputs, trace=False)
    return out
